# revision 22
# baseline (speedup 1.0000x reference)
import time

import numpy as np

import concourse.bass as bass
import concourse.mybir as mybir
from concourse import bacc
from concourse.tile import TileContext
from concourse import bass_utils
from concourse.library_config import mlp

N = 50000
P = 8
NL = N // P            # 6250 nodes per core
F_IN = 26
H = 64
L = 6
WIN = 32               # dst-window width (onehot columns)
NWIN = (NL + WIN - 1) // WIN   # 196
BASE = 25000           # gather table base row (signed int16 idx = src - BASE)
CB = 64                # blocks per gather chunk
NCHUNK_NODES = 512     # node chunk for GRU / prologue
NBLK = (NL + 127) // 128       # 49 node blocks
NCHK = (NL + NCHUNK_NODES - 1) // NCHUNK_NODES  # 13

LAST_EXEC_NS = None
_N_LAYERS = L
_DO_GATHER = True
_DO_GRU = True
_GATHER_EXT = False
_DO_SEGSUM = True

f32 = mybir.dt.float32
i16 = mybir.dt.int16
Alu = mybir.AluOpType
Act = mybir.ActivationFunctionType


def _pack_idx(lst):
    arr = np.asarray(lst, dtype=np.int16).reshape(-1, 16).T  # [16, n/16]
    return np.tile(arr, (8, 1))                              # [128, n/16]


def _preprocess_edges(edge_index):
    src = np.asarray(edge_index[0], dtype=np.int64)
    dst = np.asarray(edge_index[1], dtype=np.int64)
    core = dst // NL

    per_core = []
    E_wc = np.zeros((P, NWIN), dtype=np.int64)
    for c in range(P):
        m = core == c
        s = src[m]
        dl = dst[m] - c * NL
        w = dl // WIN
        order = np.argsort(w, kind="stable")
        s, dl, w = s[order], dl[order], w[order]
        E_wc[c] = np.bincount(w, minlength=NWIN)
        per_core.append((s, dl, w))

    K_w = np.ceil(E_wc / 128.0).astype(np.int64).max(axis=0)  # [NWIN]

    blk_win = []
    win_first = np.zeros(NWIN, dtype=np.int64)
    for w in range(NWIN):
        win_first[w] = len(blk_win)
        blk_win.extend([w] * int(K_w[w]))
    blk_win = np.asarray(blk_win, dtype=np.int64)
    NB = len(blk_win)
    nch = (NB + CB - 1) // CB

    idx_all = np.full((P, NB * 128), 1, dtype=np.int64)      # pads -> row BASE+1
    rel_all = np.full((P, NB * 128), -1.0, dtype=np.float32)
    for c in range(P):
        s, dl, w = per_core[c]
        counts = E_wc[c]
        wstart = np.concatenate([[0], np.cumsum(counts)[:-1]])
        rank = np.arange(len(w)) - wstart[w]
        slots = win_first[w] * 128 + rank
        idx_all[c, slots] = s - BASE
        rel_all[c, slots] = (dl - w * WIN).astype(np.float32)

    # ensure the final slot of every gather call has a non-negative index
    # (SWDGE trims trailing negatives); swap within the same window if needed
    call_last = {min((k + 1) * CB, NB) * 128 - 1 for k in range(nch)}
    for c in range(P):
        for ci in range(nch):
            last = min((ci + 1) * CB, NB) * 128 - 1
            if idx_all[c, last] >= 0:
                continue
            w = int(blk_win[last // 128])
            lo = int(win_first[w]) * 128
            hi = (int(win_first[w]) + int(K_w[w])) * 128
            jcand = None
            for j in range(hi - 1, lo - 1, -1):
                if j != last and j not in call_last and idx_all[c, j] >= 0:
                    jcand = j
                    break
            if jcand is None:
                raise RuntimeError("no swap partner for trailing negative idx")
            idx_all[c, [last, jcand]] = idx_all[c, [jcand, last]]
            rel_all[c, [last, jcand]] = rel_all[c, [jcand, last]]

    idx_tabs = np.empty((P, 128, NB * 8), dtype=np.int16)
    for c in range(P):
        parts = []
        for ci in range(nch):
            b0, b1 = ci * CB, min((ci + 1) * CB, NB)
            parts.append(_pack_idx(idx_all[c, b0 * 128:b1 * 128]))
        idx_tabs[c] = np.hstack(parts)

    rel_tabs = np.ascontiguousarray(
        rel_all.reshape(P, NB, 128).transpose(0, 2, 1))      # [P, 128, NB]

    return NB, nch, K_w, blk_win, win_first, idx_tabs, rel_tabs


def _build(nb, nch, K_w, blk_win, win_first):
    nc = bacc.Bacc(num_devices=P)
    xT_t = nc.dram_tensor("xT", [F_IN, NL], f32, kind="ExternalInput")
    idx_t = nc.dram_tensor("idx_t", [128, nb * 8], i16, kind="ExternalInput")
    rel_t = nc.dram_tensor("rel_t", [128, nb], f32, kind="ExternalInput")
    iota_t = nc.dram_tensor("iota_t", [128, WIN], f32, kind="ExternalInput")
    lin0_t = nc.dram_tensor("lin0_t", [F_IN, H], f32, kind="ExternalInput")
    conv_t = nc.dram_tensor("conv_t", [H, L * H], f32, kind="ExternalInput")
    ih_t = nc.dram_tensor("ih_t", [H, 3 * H], f32, kind="ExternalInput")
    hh_t = nc.dram_tensor("hh_t", [H, 3 * H], f32, kind="ExternalInput")
    bias_t = nc.dram_tensor("bias_t", [H, 4], f32, kind="ExternalInput")
    l1w_t = nc.dram_tensor("l1w_t", [H, 1], f32, kind="ExternalInput")
    l1b_t = nc.dram_tensor("l1b_t", [1, 1], f32, kind="ExternalInput")
    out_t = nc.dram_tensor("out_t", [1, NL], f32, kind="ExternalOutput")
    x1_t = nc.dram_tensor("x1_t", [NL, H], f32, kind="ExternalOutput")
    tbl_t = (nc.dram_tensor("tbl_t", [N, H], f32, kind="ExternalInput")
             if _GATHER_EXT else None)

    with TileContext(nc) as tc:
        with tc.tile_pool(name="persist", bufs=1) as pers, \
             tc.tile_pool(name="dram", bufs=1, space="DRAM") as dpool:
            nc.gpsimd.load_library(mlp)

            hT = pers.tile([H, NL], f32)
            agg = pers.tile([H, NL], f32)
            idx_sb = pers.tile([128, nb * 8], i16)
            rel_sb = pers.tile([128, nb], f32)
            iota_sb = pers.tile([128, WIN], f32)
            w0 = pers.tile([F_IN, H], f32)
            wc_ = pers.tile([H, L * H], f32)
            wih = pers.tile([H, 3 * H], f32)
            whh = pers.tile([H, 3 * H], f32)
            wb = pers.tile([H, 4], f32)
            w1 = pers.tile([H, 1], f32)
            b1sb = pers.tile([1, 1], f32)
            for sb, dr in [(idx_sb, idx_t), (rel_sb, rel_t), (iota_sb, iota_t),
                           (w0, lin0_t), (wc_, conv_t), (wih, ih_t), (whh, hh_t),
                           (wb, bias_t), (w1, l1w_t), (b1sb, l1b_t)]:
                nc.sync.dma_start(out=sb[:], in_=dr[:])
            nc.vector.memset(agg[:], 0.0)

            cc_in = dpool.tile([NL, H], f32)
            cc_out = dpool.tile([N, H], f32)

            with tc.tile_pool(name="lay", bufs=2) as lp, \
                 tc.tile_pool(name="msgp", bufs=2) as msgp, \
                 tc.tile_pool(name="ohp", bufs=2) as ohp, \
                 tc.tile_pool(name="psp", bufs=1, space="PSUM") as psp:

                # ---- prologue: x1 = sigmoid(x @ lin0_W); h = x1 ----
                with tc.tile_pool(name="xpool", bufs=1) as xpool:
                    xsb = xpool.tile([F_IN, NL], f32)
                    nc.sync.dma_start(out=xsb[:], in_=xT_t[:])
                    for j in range(NCHK):
                        c0 = j * NCHUNK_NODES
                        cw = min(NCHUNK_NODES, NL - c0)
                        ps = psp.tile([H, NCHUNK_NODES], f32, name="grx", bufs=2)
                        nc.tensor.matmul(out=ps[:, :cw], lhsT=w0[:],
                                         rhs=xsb[:, c0:c0 + cw], start=True, stop=True)
                        nc.scalar.activation(out=hT[:, c0:c0 + cw], in_=ps[:, :cw],
                                             func=Act.Sigmoid)
                    for b in range(NBLK):
                        r0 = b * 128
                        rw = min(128, NL - r0)
                        ps = psp.tile([128, H], f32, name="mps", bufs=2)
                        nc.tensor.matmul(out=ps[:rw, :], lhsT=xsb[:, r0:r0 + rw],
                                         rhs=w0[:], start=True, stop=True)
                        sb = lp.tile([128, H], f32)
                        nc.scalar.activation(out=sb[:rw, :], in_=ps[:rw, :],
                                             func=Act.Sigmoid)
                        nc.sync.dma_start(out=x1_t[r0:r0 + rw, :], in_=sb[:rw, :])

                # ---- layers ----
                for l in range(_N_LAYERS):
                    # m = h @ conv_W[l] (node-major) -> cc_in -> AllGather
                    for b in range(NBLK):
                        r0 = b * 128
                        rw = min(128, NL - r0)
                        ps = psp.tile([128, H], f32, name="mps", bufs=2)
                        nc.tensor.matmul(out=ps[:rw, :], lhsT=hT[:, r0:r0 + rw],
                                         rhs=wc_[:, l * H:(l + 1) * H],
                                         start=True, stop=True)
                        sb = lp.tile([128, H], f32)
                        nc.scalar.mul(out=sb[:rw, :], in_=ps[:rw, :], mul=1.0)
                        nc.sync.dma_start(out=cc_in[r0:r0 + rw, :], in_=sb[:rw, :])
                    nc.gpsimd.collective_compute(
                        "AllGather", Alu.bypass,
                        replica_groups=[list(range(P))],
                        ins=[cc_in.opt()], outs=[cc_out.opt()])

                    # gather + windowed segment-sum
                    win_ps = {}
                    for ci in range(nch if _DO_GATHER else 0):
                        b0, b1 = ci * CB, min((ci + 1) * CB, nb)
                        cb = b1 - b0
                        msg = msgp.tile([128, cb, H], f32)
                        gsrc = tbl_t if _GATHER_EXT else cc_out
                        nc.gpsimd.dma_gather(msg[:], gsrc[BASE:, :],
                                             idx_sb[:, b0 * 8:b0 * 8 + cb * 8],
                                             cb * 128, cb * 128, H,
                                             single_packet=False)
                        oh = ohp.tile([128, cb * WIN], f32)
                        for k in range(cb):
                            gb = b0 + k
                            nc.vector.tensor_tensor(
                                out=oh[:, k * WIN:(k + 1) * WIN],
                                in0=rel_sb[:, gb:gb + 1].to_broadcast([128, WIN]),
                                in1=iota_sb[:], op=Alu.is_equal)
                        for k in range(cb if _DO_SEGSUM else 0):
                            gb = b0 + k
                            w = int(blk_win[gb])
                            kk = gb - int(win_first[w])
                            first = kk == 0
                            last = kk == int(K_w[w]) - 1
                            if first:
                                win_ps[w] = psp.tile(
                                    [H, WIN], f32, name="wps", bufs=2)
                            ps = win_ps[w]
                            nc.tensor.matmul(out=ps[:], lhsT=msg[:, k, :],
                                             rhs=oh[:, k * WIN:(k + 1) * WIN],
                                             start=first, stop=last)
                            if last:
                                wdt = min(WIN, NL - w * WIN)
                                nc.scalar.mul(out=agg[:, w * WIN:w * WIN + wdt],
                                              in_=ps[:, :wdt], mul=1.0)
                                del win_ps[w]

                    # GRU: h = GRUCell(agg, h)
                    for j in range(NCHK if _DO_GRU else 0):
                        c0 = j * NCHUNK_NODES
                        cw = min(NCHUNK_NODES, NL - c0)
                        a_ch = agg[:, c0:c0 + cw]
                        h_ch = hT[:, c0:c0 + cw]
                        ps_r = psp.tile([H, NCHUNK_NODES], f32, name="grx", bufs=2)
                        nc.tensor.matmul(out=ps_r[:, :cw], lhsT=wih[:, 0:H],
                                         rhs=a_ch, start=True, stop=False)
                        nc.tensor.matmul(out=ps_r[:, :cw], lhsT=whh[:, 0:H],
                                         rhs=h_ch, start=False, stop=True)
                        r_sb = lp.tile([H, NCHUNK_NODES], f32)
                        nc.scalar.activation(out=r_sb[:, :cw], in_=ps_r[:, :cw],
                                             func=Act.Sigmoid, bias=wb[:, 0:1])
                        ps_z = psp.tile([H, NCHUNK_NODES], f32, name="grx", bufs=2)
                        nc.tensor.matmul(out=ps_z[:, :cw], lhsT=wih[:, H:2 * H],
                                         rhs=a_ch, start=True, stop=False)
                        nc.tensor.matmul(out=ps_z[:, :cw], lhsT=whh[:, H:2 * H],
                                         rhs=h_ch, start=False, stop=True)
                        z_sb = lp.tile([H, NCHUNK_NODES], f32)
                        nc.scalar.activation(out=z_sb[:, :cw], in_=ps_z[:, :cw],
                                             func=Act.Sigmoid, bias=wb[:, 1:2])
                        ps_in = psp.tile([H, NCHUNK_NODES], f32, name="gn", bufs=2)
                        nc.tensor.matmul(out=ps_in[:, :cw], lhsT=wih[:, 2 * H:3 * H],
                                         rhs=a_ch, start=True, stop=True)
                        ps_hn = psp.tile([H, NCHUNK_NODES], f32, name="gn", bufs=2)
                        nc.tensor.matmul(out=ps_hn[:, :cw], lhsT=whh[:, 2 * H:3 * H],
                                         rhs=h_ch, start=True, stop=True)
                        ta = lp.tile([H, NCHUNK_NODES], f32)
                        nc.vector.tensor_scalar(out=ta[:, :cw], in0=ps_hn[:, :cw],
                                                scalar1=wb[:, 3:4], scalar2=None,
                                                op0=Alu.add)
                        nc.vector.tensor_tensor(out=ta[:, :cw], in0=ta[:, :cw],
                                                in1=r_sb[:, :cw], op=Alu.mult)
                        nc.vector.tensor_tensor(out=ta[:, :cw], in0=ta[:, :cw],
                                                in1=ps_in[:, :cw], op=Alu.add)
                        n_sb = lp.tile([H, NCHUNK_NODES], f32)
                        nc.scalar.activation(out=n_sb[:, :cw], in_=ta[:, :cw],
                                             func=Act.Tanh, bias=wb[:, 2:3])
                        tb = lp.tile([H, NCHUNK_NODES], f32)
                        nc.vector.tensor_tensor(out=tb[:, :cw], in0=h_ch,
                                                in1=n_sb[:, :cw], op=Alu.subtract)
                        nc.vector.tensor_tensor(out=tb[:, :cw], in0=tb[:, :cw],
                                                in1=z_sb[:, :cw], op=Alu.mult)
                        nc.vector.tensor_tensor(out=h_ch, in0=n_sb[:, :cw],
                                                in1=tb[:, :cw], op=Alu.add)

                # ---- epilogue: out = relu(h) @ lin1_W + lin1_b ----
                for j in range(NCHK):
                    c0 = j * NCHUNK_NODES
                    cw = min(NCHUNK_NODES, NL - c0)
                    rl = lp.tile([H, NCHUNK_NODES], f32)
                    nc.scalar.activation(out=rl[:, :cw], in_=hT[:, c0:c0 + cw],
                                         func=Act.Relu)
                    ps_o = psp.tile([H, NCHUNK_NODES], f32, name="grx", bufs=2)
                    nc.tensor.matmul(out=ps_o[0:1, :cw], lhsT=w1[:], rhs=rl[:, :cw],
                                     start=True, stop=True)
                    o_sb = lp.tile([1, NCHUNK_NODES], f32)
                    nc.vector.tensor_scalar(out=o_sb[:, :cw], in0=ps_o[0:1, :cw],
                                            scalar1=b1sb[0:1, 0:1], scalar2=None,
                                            op0=Alu.add)
                    nc.sync.dma_start(out=out_t[0:1, c0:c0 + cw], in_=o_sb[:, :cw])
    nc.compile()
    return nc


def kernel(x, edge_index, lin0_W, conv_W, W_ih, W_hh, b_ih, b_hh, lin1_W, lin1_b):
    global LAST_EXEC_NS
    x = np.asarray(x, dtype=np.float32)
    lin0_W = np.asarray(lin0_W, dtype=np.float32)
    conv_W = np.asarray(conv_W, dtype=np.float32)
    W_ih = np.asarray(W_ih, dtype=np.float32)
    W_hh = np.asarray(W_hh, dtype=np.float32)
    b_ih = np.asarray(b_ih, dtype=np.float32)
    b_hh = np.asarray(b_hh, dtype=np.float32)
    lin1_W = np.asarray(lin1_W, dtype=np.float32)
    lin1_b = np.asarray(lin1_b, dtype=np.float32)

    NB, nch, K_w, blk_win, win_first, idx_tabs, rel_tabs = _preprocess_edges(edge_index)
    nc = _build(NB, nch, K_w, blk_win, win_first)

    convcat = np.ascontiguousarray(np.concatenate([conv_W[l] for l in range(L)], axis=1))
    gruih = np.ascontiguousarray(W_ih.T)
    gruhh = np.ascontiguousarray(W_hh.T)
    biases = np.ascontiguousarray(np.stack(
        [b_ih[0:H] + b_hh[0:H], b_ih[H:2 * H] + b_hh[H:2 * H],
         b_ih[2 * H:3 * H], b_hh[2 * H:3 * H]], axis=1))
    iota = np.tile(np.arange(WIN, dtype=np.float32), (128, 1))

    rng_tbl = (np.random.default_rng(3).standard_normal((N, H)).astype(np.float32)
               if _GATHER_EXT else None)
    in_maps = []
    for c in range(P):
        in_maps.append(dict(
            xT=np.ascontiguousarray(x[c * NL:(c + 1) * NL].T),
            idx_t=idx_tabs[c],
            rel_t=rel_tabs[c],
            iota_t=iota,
            lin0_t=lin0_W,
            conv_t=convcat,
            ih_t=gruih,
            hh_t=gruhh,
            bias_t=biases,
            l1w_t=lin1_W,
            l1b_t=lin1_b.reshape(1, 1),
        ))
        if _GATHER_EXT:
            in_maps[-1]["tbl_t"] = rng_tbl

    t0 = time.perf_counter()
    res = bass_utils.run_bass_kernel_spmd(nc, in_maps, core_ids=list(range(P)))
    t1 = time.perf_counter()
    LAST_EXEC_NS = int((t1 - t0) * 1e9)

    out = np.concatenate([res.results[c]["out_t"][0] for c in range(P)])
    x1 = np.concatenate([res.results[c]["x1_t"] for c in range(P)], axis=0)
    return out.astype(np.float32), x1.astype(np.float32)
